# revision 14
# baseline (speedup 1.0000x reference)
"""BSMatchStar Trainium2 kernel (final, 643.4us on 8 cores; baseline 984us).

out = (a | (((a&b) +_brev b) ^ b)) -- bitstream MatchStar via a 2^29-bit
big-integer addition over per-byte bit-reversed operands.

Per core (1/8 contiguous slice, 8 tiles of [128, 2048] uint32 limbs),
processed in software-pipelined waves of W=2 tiles:
  Phase 1: X=brev(a&b), Y=brev(b) by SWAR (per level: A=(x<<s)&mL [ts 2x],
    Braw=x>>s [ts 2x], y=(Braw&mR)|A [scalar-tensor-tensor 1x]); S=X+Y on
    GPSIMD (the only engine with an exact uint32 add; DVE add saturates);
    limb flags on the otherwise-idle Scalar/ACT engine via the Sign
    activation: kill k~=sign(-(~S)) in {0,-1} (accum_out -> per-row
    all-propagate), generate g=sign((X|(Y&~S)) & 0x80000000); row-local
    carry scan with recurrence state = max(state + k~, g).
  Stitch per wave: row (G,P) flags bounce through DRAM to transpose into
    one [1, W*128] stream; one seeded scan produces every row's carry-in
    seed (decoupled lookback); stitch DMA-out/readback halves are split
    and overlapped with neighbouring waves' compute.
  Phase 2: seeded scan -> limb carries ci; S' = S + ci (GPSIMD);
    out = (brev(S') ^ b) | a  (b stays SBUF-resident; a re-streamed, and
    the next wave's inputs are prefetched from phase 2 as buffers free up
    so input DMA bursts never stall the DVE).
  Cross-core carry resolved host-side (decoupled lookback; O(1) bytes of
  fixup per core boundary).

Engine discipline (the load-bearing discovery): GPSIMD shares SBUF ports
with the DVE, so sustained GPSIMD streaming halves DVE throughput (the
984us baseline and a fully-overlapped 2-engine variant measured the same
span). All elementwise work therefore runs on the DVE at 2x_2P perf
modes; GPSIMD does only the two integer adds per tile; flag extraction
rides the ACT engine. Pair emission is skewed so one tile's GPSIMD add
overlaps the other tile's DVE work.
"""
import sys
sys.path.insert(0, "/opt/trn_rl_repo")

import numpy as np

N_BYTES = 67_108_864
N_CORES = 8
P = 128
F = 2048
WORDS_PER_CORE = N_BYTES // 4 // N_CORES
T = WORDS_PER_CORE // (P * F)  # 8

_BREV = np.array([int(f"{i:08b}"[::-1], 2) for i in range(256)], dtype=np.uint8)

_cache = {}

LVLS = [(4, 0xF0F0F0F0, 0x0F0F0F0F),
        (2, 0xCCCCCCCC, 0x33333333),
        (1, 0xAAAAAAAA, 0x55555555)]


def _build(n_tiles, f):
    import concourse.bacc as bacc
    import concourse.tile as tile
    import concourse.mybir as mybir
    import contextlib

    AOT = mybir.AluOpType
    dt = mybir.dt

    nc = bacc.Bacc("TRN2", target_bir_lowering=False, debug=False)

    d_a = nc.dram_tensor("a", [n_tiles, P, f], dt.uint32, kind="ExternalInput")
    d_b = nc.dram_tensor("b", [n_tiles, P, f], dt.uint32, kind="ExternalInput")
    d_o = nc.dram_tensor("o", [n_tiles, P, f], dt.uint32, kind="ExternalOutput")
    d_agg = nc.dram_tensor("agg", [1, 2], dt.float32, kind="ExternalOutput")
    d_gp = nc.dram_tensor("scr_gp", [2, n_tiles, P], dt.int8)
    d_seed = nc.dram_tensor("scr_seed", [1, n_tiles, P], dt.float32)

    def ts(out, in0, s1, s2, op0, op1=None):
        if op1 is not None:
            nc.vector.tensor_scalar(out, in0, s1, s2, op0, op1)
        else:
            nc.vector.tensor_scalar(out, in0, s1, s2, op0)

    def tt(out, a, b, op):
        nc.vector.tensor_tensor(out, a, b, op)

    def stt(out, in0, imm, in1, op0, op1):
        nc.vector.add_instruction(mybir.InstTensorScalarPtr(
            name=nc.get_next_instruction_name(), is_scalar_tensor_tensor=True,
            op0=op0, op1=op1,
            ins=[nc.vector.lower_ap(in0),
                 mybir.ImmediateValue(dtype=dt.uint32, value=imm),
                 nc.vector.lower_ap(in1)],
            outs=[nc.vector.lower_ap(out)]))

    def gp_add(out, a, b):
        nc.gpsimd.tensor_tensor(out, a, b, AOT.add)

    with tile.TileContext(nc) as tc, contextlib.ExitStack() as ctx:
        pool = ctx.enter_context(tc.tile_pool(name="sb", bufs=1))

        W = 2
        n_waves = n_tiles // W
        SLOTS = 2 * W  # two waves of resident state

        S_t = [pool.tile([P, f], dt.uint32, tag=f"S{s}", name=f"S{s}")
               for s in range(SLOTS)]
        b_t = [pool.tile([P, f], dt.uint32, tag=f"b{s}", name=f"b{s}")
               for s in range(SLOTS)]
        k8_t = [pool.tile([P, f], dt.int8, tag=f"k8{s}", name=f"k8{s}")
                for s in range(SLOTS)]
        g01_t = [pool.tile([P, f], dt.int8, tag=f"g01{s}", name=f"g01{s}")
                 for s in range(SLOTS)]

        def scratch(tag, t, dtype=dt.uint32, shape=None):
            return pool.tile(shape or [P, f], dtype, tag=f"{tag}_{t % 2}",
                             name=f"{tag}_{t % 2}")

        gpw = [pool.tile([P, 2, W], dt.int8, tag=f"gpw_{k % 2}",
                         name=f"gpw_{k % 2}") for k in range(2)]
        seeds_p = [pool.tile([P, 1], dt.float32, tag=f"seeds_{i}",
                             name=f"seeds_{i}") for i in range(2)]
        gt_p = [pool.tile([1, W * P], dt.int8, tag=f"gt_{i}", name=f"gt_{i}")
                for i in range(2)]
        a_in = [pool.tile([P, f], dt.uint32, tag=f"ain_{i}", name=f"ain_{i}")
                for i in range(2)]
        pt_p = [pool.tile([1, W * P], dt.int8, tag=f"pt_{i}", name=f"pt_{i}")
                for i in range(2)]

        def brev_chain(t, src, final):
            """3 swap levels; mids cycle through the 't0' scratch tag."""
            x = src
            for li, (sh, mL, mR) in enumerate(LVLS):
                A = scratch("A", t)
                Braw = scratch("Braw", t)
                y = final if li == 2 else scratch("t0", t)
                yield lambda x=x, sh=sh, mL=mL, A=A: ts(
                    A[:], x[:], sh, mL,
                    AOT.logical_shift_left, AOT.bitwise_and)
                yield lambda x=x, sh=sh, Braw=Braw: ts(
                    Braw[:], x[:], sh, None, AOT.logical_shift_right)
                yield lambda y=y, Braw=Braw, mR=mR, A=A: stt(
                    y[:], Braw[:], mR, A[:],
                    AOT.bitwise_and, AOT.bitwise_or)
                x = y

        # ---------------- phase 1 ------------------------------------------
        def phase1_ops(t):
            s = t % SLOTS
            k = (t // W) % 2
            a_t = a_in[t % 2]
            if t < 2 * W:          # later waves are prefetched from phase 2
                yield lambda: nc.sync.dma_start(a_t[:], d_a[t])
                yield lambda: nc.sync.dma_start(b_t[s][:], d_b[t])

            t0 = scratch("t0", t)
            yield lambda: tt(t0[:], a_t[:], b_t[s][:], AOT.bitwise_and)

            X = scratch("a", t)     # a dead after t0; X lives to gmsb
            Y = pool.tile([P, f], dt.uint32, tag=f"ci_{t % 2}",
                          name=f"Y_{t % 2}")   # ci tag free during phase 1
            yield from brev_chain(t, t0, X)
            yield from brev_chain(t, b_t[s], Y)

            yield lambda: gp_add(S_t[s][:], X[:], Y[:])

            nots = scratch("t0", t)
            yield lambda: ts(nots[:], S_t[s][:], 0xFFFFFFFF, None,
                             AOT.bitwise_xor)
            # k~ = sign(-nots) in {0,-1} on the (otherwise idle) ACT engine;
            # accum gives -#non-propagate per row -> P_row == (acc == 0)
            accrow = scratch("accrow", t, dt.float32, [P, 1])
            yield lambda: nc.scalar.activation(
                k8_t[s][:], nots[:], mybir.ActivationFunctionType.Sign,
                scale=-1.0, accum_out=accrow[:])
            n1m = scratch("Braw", t)
            yield lambda: stt(n1m[:], nots[:], 0x80000000, Y[:],
                              AOT.bitwise_and, AOT.bitwise_and)
            gmsb = scratch("A", t)
            yield lambda: stt(gmsb[:], X[:], 0x80000000, n1m[:],
                              AOT.bitwise_and, AOT.bitwise_or)
            yield lambda: nc.scalar.activation(
                g01_t[s][:], gmsb[:], mybir.ActivationFunctionType.Sign)

            lc8 = scratch("lc8", t, dt.int8)
            yield lambda: nc.vector.tensor_tensor_scan(
                lc8[:], k8_t[s][:], g01_t[s][:], 0.0, AOT.add, AOT.max)
            yield lambda: nc.vector.tensor_copy(gpw[k][:, 0, t % W:t % W + 1],
                                                lc8[:, f - 1:f])
            yield lambda: nc.vector.tensor_scalar(
                gpw[k][:, 1, t % W:t % W + 1], accrow[:], 0.0, None,
                AOT.is_equal)

        # ---------------- per-wave stitch ----------------------------------
        carry = pool.tile([1, 1], dt.float32, tag="carry_init")
        nc.vector.memset(carry[:], 0.0)
        pcore = pool.tile([1, 1], dt.float32, tag="pcore_init")
        nc.vector.memset(pcore[:], 1.0)

        def stitch_out(kw):
            k = kw % 2
            w0 = kw * W
            nc.sync.dma_start(d_gp[0, w0:w0 + W].rearrange("t p -> p t"),
                              gpw[k][:, 0, :])
            nc.sync.dma_start(d_gp[1, w0:w0 + W].rearrange("t p -> p t"),
                              gpw[k][:, 1, :])
            nc.sync.dma_start(
                gt_p[k][:],
                d_gp[0:1, w0:w0 + W].rearrange("one t p -> one (t p)"))
            nc.sync.dma_start(
                pt_p[k][:],
                d_gp[1:2, w0:w0 + W].rearrange("one t p -> one (t p)"))

        def stitch_in(kw):
            nonlocal carry, pcore
            k = kw % 2
            w0 = kw * W
            gt = gt_p[k]
            pt = pt_p[k]

            scw = pool.tile([1, W * P], dt.float32, tag=f"scw_{k}",
                            name=f"scw_{k}")
            nc.vector.tensor_tensor_scan(scw[:], pt[:], gt[:], carry[:],
                                         AOT.mult, AOT.max)
            seeds_row = pool.tile([1, W * P], dt.float32, tag=f"sr_{k}",
                                  name=f"sr_{k}")
            nc.vector.tensor_copy(seeds_row[0:1, 0:1], carry[:])
            nc.vector.tensor_copy(seeds_row[0:1, 1:], scw[0:1, :W * P - 1])
            nc.sync.dma_start(
                d_seed[0:1, w0:w0 + W, :].rearrange("one t p -> one (t p)"),
                seeds_row[:])

            for tt_ in range(w0, w0 + W):
                nc.sync.dma_start(
                    seeds_p[tt_ % 2][:],
                    d_seed[0, tt_:tt_ + 1, :].rearrange("one p -> p one"))

            ncarry = pool.tile([1, 1], dt.float32, tag=f"carry{kw}",
                               name=f"carry{kw}")
            nc.vector.tensor_copy(ncarry[:], scw[0:1, W * P - 1:W * P])
            carry = ncarry
            ptile = pool.tile([1, 1], dt.float32, tag=f"ptile{kw}",
                              name=f"ptile{kw}")
            nc.vector.tensor_reduce(ptile[:], pt[:], mybir.AxisListType.X,
                                    AOT.min)
            npcore = pool.tile([1, 1], dt.float32, tag=f"pcore{kw}",
                               name=f"pcore{kw}")
            nc.vector.tensor_tensor(npcore[:], pcore[:], ptile[:], AOT.min)
            pcore = npcore

        # ---------------- phase 2 (head / tail split) ----------------------
        p2h = {}

        def phase2_head(t):
            s = t % SLOTS
            seeds = seeds_p[t % 2]

            ci = scratch("ci", t, dt.uint32, [P, f + 1])
            yield lambda: nc.vector.tensor_tensor_scan(
                ci[:, 1:f + 1], k8_t[s][:], g01_t[s][:], seeds[:],
                AOT.add, AOT.max)
            yield lambda: nc.vector.tensor_copy(ci[:, 0:1], seeds[:])

            sp = scratch("t0", t)
            p2h[t] = sp
            yield lambda: gp_add(sp[:], S_t[s][:], ci[:, 0:f])
            yield lambda: nc.sync.dma_start(a_in[t % 2][:], d_a[t])

        def phase2_tail(t):
            s = t % SLOTS
            sp = p2h.pop(t)
            a2 = a_in[t % 2]

            wb = pool.tile([P, f], dt.uint32, tag=f"ci_{t % 2}",
                           name=f"wb_{t % 2}")
            yield from brev_chain(t, sp, wb)

            t1 = scratch("A", t)
            yield lambda: tt(t1[:], wb[:], b_t[s][:], AOT.bitwise_xor)
            if t + 2 * W < n_tiles:    # b slot free: prefetch wave kw+2
                yield lambda: nc.sync.dma_start(b_t[s][:], d_b[t + 2 * W])
            o_t = scratch("t0", t)
            if t + 2 * W < n_tiles:
                yield lambda: tt(o_t[:], t1[:], a2[:], AOT.bitwise_or)
                # a scratch free: prefetch wave kw+2
                yield lambda: nc.sync.dma_start(a_in[t % 2][:],
                                                d_a[t + 2 * W])
                yield lambda: nc.sync.dma_start(d_o[t], o_t[:])
            else:
                # last wave: drain the output in halves to shorten the tail
                h = f // 2
                yield lambda: tt(o_t[:, :h], t1[:, :h], a2[:, :h],
                                 AOT.bitwise_or)
                yield lambda: nc.sync.dma_start(d_o[t, :, 0:h], o_t[:, :h])
                yield lambda: tt(o_t[:, h:], t1[:, h:], a2[:, h:],
                                 AOT.bitwise_or)
                yield lambda: nc.sync.dma_start(d_o[t, :, h:f], o_t[:, h:])

        # ---------------- pipelined emission -------------------------------
        def emit_pairs(gen_fn, kw, skew=7):
            # Skewed pairwise interleave: tile t runs `skew` ops ahead of
            # t+1 so their GPSIMD adds don't align (DVE fills the gap).
            for t in range(kw * W, (kw + 1) * W, 2):
                gens = [gen_fn(t), gen_fn(t + 1)]
                done = [False, False]
                for _ in range(skew):
                    try:
                        next(gens[0])()
                    except StopIteration:
                        done[0] = True
                        break
                while not all(done):
                    for i, g in enumerate(gens):
                        if not done[i]:
                            try:
                                next(g)()
                            except StopIteration:
                                done[i] = True

        emit_pairs(phase1_ops, 0)
        stitch_out(0)
        emit_pairs(phase1_ops, 1)
        stitch_out(1)
        stitch_in(0)
        for kw in range(n_waves):
            emit_pairs(phase2_head, kw, skew=2)
            if kw + 1 < n_waves:
                stitch_in(kw + 1)     # DVE work that fills the S'-add hole
            emit_pairs(phase2_tail, kw)
            if kw + 2 < n_waves:
                emit_pairs(phase1_ops, kw + 2)
                stitch_out(kw + 2)

        agg = pool.tile([1, 2], dt.float32, tag="agg")
        nc.vector.tensor_copy(agg[0:1, 0:1], carry[:])
        nc.vector.tensor_copy(agg[0:1, 1:2], pcore[:])
        nc.sync.dma_start(d_agg[:], agg[:])

    nc.compile()
    return nc


def _get_nc(n_tiles, f):
    key = (n_tiles, f)
    if key not in _cache:
        _cache[key] = _build(n_tiles, f)
    return _cache[key]


def run_sharded(a_u8, b_u8, n_cores=N_CORES, f=F, **spmd_kwargs):
    from concourse import bass_utils

    n = a_u8.size
    words = n // 4
    wpc = words // n_cores
    n_tiles = wpc // (P * f)
    assert n_tiles * P * f == wpc, (n, n_cores, f)

    a32 = a_u8.view(np.uint32).reshape(n_cores, n_tiles, P, f)
    b32 = b_u8.view(np.uint32).reshape(n_cores, n_tiles, P, f)

    nc = _get_nc(n_tiles, f)
    in_maps = [{"a": np.ascontiguousarray(a32[c]),
                "b": np.ascontiguousarray(b32[c])}
               for c in range(n_cores)]
    res = bass_utils.run_bass_kernel_spmd(nc, in_maps,
                                          core_ids=list(range(n_cores)),
                                          **spmd_kwargs)
    outs = [r["o"] for r in res.results]
    aggs = [(float(r["agg"][0, 0]), float(r["agg"][0, 1]))
            for r in res.results]
    out = np.concatenate([o.reshape(-1).view(np.uint8) for o in outs])
    return out, aggs, res


def _fixup_boundaries(out, a_u8, b_u8, aggs, n_cores):
    """Resolve the cross-core carry (decoupled lookback, host side)."""
    n = out.size
    csize = n // n_cores
    c_in = 0
    for c in range(n_cores):
        g_c = 1 if aggs[c][0] != 0.0 else 0
        p_c = 1 if aggs[c][1] != 0.0 else 0
        if c_in == 1:
            st = c * csize
            en = st + csize
            i = st
            done = False
            while i < en and not done:
                j = min(i + 65536, en)
                aa = a_u8[i:j]
                bb = b_u8[i:j]
                raw = _BREV[aa & bb].astype(np.int32) + _BREV[bb].astype(np.int32)
                prop = raw == 255
                if prop.all():
                    out[i:j] = aa | bb
                    i = j
                    continue
                k = int(np.argmin(prop))
                out[i:i + k] = aa[:k] | bb[:k]
                idx = i + k
                new_s = (int(raw[k]) + 1) & 0xFF
                out[idx] = (int(_BREV[new_s]) ^ int(b_u8[idx])) | int(a_u8[idx])
                done = True
        c_in = g_c | (p_c & c_in)
    return out


def kernel(a, b):
    assert a.dtype == np.uint8 and b.dtype == np.uint8 and a.size == N_BYTES
    out, aggs, _ = run_sharded(a, b)
    out = _fixup_boundaries(out, a, b, aggs, N_CORES)
    return out


# revision 18
# speedup vs baseline: 1.2029x; 1.2029x over previous
"""BSMatchStar Trainium2 kernel (final, 643.4us on 8 cores; baseline 984us).

out = (a | (((a&b) +_brev b) ^ b)) -- bitstream MatchStar via a 2^29-bit
big-integer addition over per-byte bit-reversed operands.

Per core (1/8 contiguous slice, 8 tiles of [128, 2048] uint32 limbs),
processed in software-pipelined waves of W=2 tiles:
  Phase 1: X=brev(a&b), Y=brev(b) by SWAR (per level: A=(x<<s)&mL [ts 2x],
    Braw=x>>s [ts 2x], y=(Braw&mR)|A [scalar-tensor-tensor 1x]); S=X+Y on
    GPSIMD (the only engine with an exact uint32 add; DVE add saturates);
    limb flags on the otherwise-idle Scalar/ACT engine via the Sign
    activation: kill k~=sign(-(~S)) in {0,-1} (accum_out -> per-row
    all-propagate), generate g=sign((X|(Y&~S)) & 0x80000000); row-local
    carry scan with recurrence state = max(state + k~, g).
  Stitch per wave: row (G,P) flags bounce through DRAM to transpose into
    one [1, W*128] stream; one seeded scan produces every row's carry-in
    seed (decoupled lookback); stitch DMA-out/readback halves are split
    and overlapped with neighbouring waves' compute.
  Phase 2: seeded scan -> limb carries ci; S' = S + ci (GPSIMD);
    out = (brev(S') ^ b) | a  (b stays SBUF-resident; a re-streamed, and
    the next wave's inputs are prefetched from phase 2 as buffers free up
    so input DMA bursts never stall the DVE).
  Cross-core carry resolved host-side (decoupled lookback; O(1) bytes of
  fixup per core boundary).

Engine discipline (the load-bearing discovery): GPSIMD shares SBUF ports
with the DVE, so sustained GPSIMD streaming halves DVE throughput (the
984us baseline and a fully-overlapped 2-engine variant measured the same
span). All elementwise work therefore runs on the DVE at 2x_2P perf
modes; GPSIMD does only the two integer adds per tile; flag extraction
rides the ACT engine. Pair emission is skewed so one tile's GPSIMD add
overlaps the other tile's DVE work.
"""
import sys
sys.path.insert(0, "/opt/trn_rl_repo")

import numpy as np

N_BYTES = 67_108_864
N_CORES = 8
P = 128
F = 2048
WORDS_PER_CORE = N_BYTES // 4 // N_CORES
T = WORDS_PER_CORE // (P * F)  # 8

_BREV = np.array([int(f"{i:08b}"[::-1], 2) for i in range(256)], dtype=np.uint8)

_cache = {}

LVLS = [(4, 0xF0F0F0F0, 0x0F0F0F0F),
        (2, 0xCCCCCCCC, 0x33333333),
        (1, 0xAAAAAAAA, 0x55555555)]


def _build(n_tiles, f):
    import concourse.bacc as bacc
    import concourse.tile as tile
    import concourse.mybir as mybir
    import contextlib

    AOT = mybir.AluOpType
    dt = mybir.dt

    nc = bacc.Bacc("TRN2", target_bir_lowering=False, debug=False)

    d_a = nc.dram_tensor("a", [n_tiles, P, f], dt.uint32, kind="ExternalInput")
    d_b = nc.dram_tensor("b", [n_tiles, P, f], dt.uint32, kind="ExternalInput")
    d_o = nc.dram_tensor("o", [n_tiles, P, f], dt.uint32, kind="ExternalOutput")
    d_agg = nc.dram_tensor("agg", [1, 2], dt.float32, kind="ExternalOutput")
    d_gp = nc.dram_tensor("scr_gp", [2, n_tiles, P], dt.int8)
    d_seed = nc.dram_tensor("scr_seed", [1, n_tiles, P], dt.float32)

    def ts(out, in0, s1, s2, op0, op1=None):
        if op1 is not None:
            nc.vector.tensor_scalar(out, in0, s1, s2, op0, op1)
        else:
            nc.vector.tensor_scalar(out, in0, s1, s2, op0)

    def tt(out, a, b, op):
        nc.vector.tensor_tensor(out, a, b, op)

    def stt(out, in0, imm, in1, op0, op1):
        nc.vector.add_instruction(mybir.InstTensorScalarPtr(
            name=nc.get_next_instruction_name(), is_scalar_tensor_tensor=True,
            op0=op0, op1=op1,
            ins=[nc.vector.lower_ap(in0),
                 mybir.ImmediateValue(dtype=dt.uint32, value=imm),
                 nc.vector.lower_ap(in1)],
            outs=[nc.vector.lower_ap(out)]))

    def gp_add(out, a, b):
        nc.gpsimd.tensor_tensor(out, a, b, AOT.add)

    with tile.TileContext(nc) as tc, contextlib.ExitStack() as ctx:
        pool = ctx.enter_context(tc.tile_pool(name="sb", bufs=1))

        W = 2
        n_waves = n_tiles // W
        SLOTS = 2 * W  # two waves of resident state

        S_t = [pool.tile([P, f], dt.uint32, tag=f"S{s}", name=f"S{s}")
               for s in range(SLOTS)]
        b_t = [pool.tile([P, f], dt.uint32, tag=f"b{s}", name=f"b{s}")
               for s in range(SLOTS)]
        k8_t = [pool.tile([P, f], dt.int8, tag=f"k8{s}", name=f"k8{s}")
                for s in range(SLOTS)]
        g01_t = [pool.tile([P, f], dt.int8, tag=f"g01{s}", name=f"g01{s}")
                 for s in range(SLOTS)]

        def scratch(tag, t, dtype=dt.uint32, shape=None):
            return pool.tile(shape or [P, f], dtype, tag=f"{tag}_{t % 2}",
                             name=f"{tag}_{t % 2}")

        gpw = [pool.tile([P, 2, W], dt.int8, tag=f"gpw_{k % 2}",
                         name=f"gpw_{k % 2}") for k in range(2)]
        seeds_p = [pool.tile([P, 1], dt.float32, tag=f"seeds_{i}",
                             name=f"seeds_{i}") for i in range(2)]
        gt_p = [pool.tile([1, W * P], dt.int8, tag=f"gt_{i}", name=f"gt_{i}")
                for i in range(2)]
        a_in = [pool.tile([P, f], dt.uint32, tag=f"ain_{i}", name=f"ain_{i}")
                for i in range(2)]
        pt_p = [pool.tile([1, W * P], dt.int8, tag=f"pt_{i}", name=f"pt_{i}")
                for i in range(2)]

        def brev_chain(t, src, final):
            """3 swap levels; mids cycle through the 't0' scratch tag."""
            x = src
            for li, (sh, mL, mR) in enumerate(LVLS):
                A = scratch("A", t)
                Braw = scratch("Braw", t)
                y = final if li == 2 else scratch("t0", t)
                yield lambda x=x, sh=sh, mL=mL, A=A: ts(
                    A[:], x[:], sh, mL,
                    AOT.logical_shift_left, AOT.bitwise_and)
                yield lambda x=x, sh=sh, Braw=Braw: ts(
                    Braw[:], x[:], sh, None, AOT.logical_shift_right)
                yield lambda y=y, Braw=Braw, mR=mR, A=A: stt(
                    y[:], Braw[:], mR, A[:],
                    AOT.bitwise_and, AOT.bitwise_or)
                x = y

        # ---------------- phase 1 ------------------------------------------
        def phase1_ops(t):
            s = t % SLOTS
            k = (t // W) % 2
            a_t = a_in[t % 2]
            if t < 2 * W:          # later waves are prefetched from phase 2
                yield lambda: nc.sync.dma_start(a_t[:], d_a[t])
                yield lambda: nc.sync.dma_start(b_t[s][:], d_b[t])

            t0 = scratch("t0", t)
            yield lambda: tt(t0[:], a_t[:], b_t[s][:], AOT.bitwise_and)

            X = scratch("a", t)     # a dead after t0; X lives to gmsb
            Y = pool.tile([P, f], dt.uint32, tag=f"ci_{t % 2}",
                          name=f"Y_{t % 2}")   # ci tag free during phase 1
            yield from brev_chain(t, t0, X)
            yield from brev_chain(t, b_t[s], Y)

            yield lambda: gp_add(S_t[s][:], X[:], Y[:])

            nots = scratch("t0", t)
            yield lambda: ts(nots[:], S_t[s][:], 0xFFFFFFFF, None,
                             AOT.bitwise_xor)
            # k~ = sign(-nots) in {0,-1} on the (otherwise idle) ACT engine;
            # accum gives -#non-propagate per row -> P_row == (acc == 0)
            accrow = scratch("accrow", t, dt.float32, [P, 1])
            yield lambda: nc.scalar.activation(
                k8_t[s][:], nots[:], mybir.ActivationFunctionType.Sign,
                scale=-1.0, accum_out=accrow[:])
            n1m = scratch("Braw", t)
            yield lambda: stt(n1m[:], nots[:], 0x80000000, Y[:],
                              AOT.bitwise_and, AOT.bitwise_and)
            gmsb = scratch("A", t)
            yield lambda: stt(gmsb[:], X[:], 0x80000000, n1m[:],
                              AOT.bitwise_and, AOT.bitwise_or)
            yield lambda: nc.scalar.activation(
                g01_t[s][:], gmsb[:], mybir.ActivationFunctionType.Sign)

            lc8 = scratch("lc8", t, dt.int8)
            yield lambda: nc.vector.tensor_tensor_scan(
                lc8[:], k8_t[s][:], g01_t[s][:], 0.0, AOT.add, AOT.max)
            yield lambda: nc.vector.tensor_copy(gpw[k][:, 0, t % W:t % W + 1],
                                                lc8[:, f - 1:f])
            yield lambda: nc.vector.tensor_scalar(
                gpw[k][:, 1, t % W:t % W + 1], accrow[:], 0.0, None,
                AOT.is_equal)

        # ---------------- per-wave stitch ----------------------------------
        carry = pool.tile([1, 1], dt.float32, tag="carry_init")
        nc.vector.memset(carry[:], 0.0)
        pcore = pool.tile([1, 1], dt.float32, tag="pcore_init")
        nc.vector.memset(pcore[:], 1.0)

        def stitch_out(kw):
            k = kw % 2
            w0 = kw * W
            nc.sync.dma_start(d_gp[0, w0:w0 + W].rearrange("t p -> p t"),
                              gpw[k][:, 0, :])
            nc.sync.dma_start(d_gp[1, w0:w0 + W].rearrange("t p -> p t"),
                              gpw[k][:, 1, :])
            nc.sync.dma_start(
                gt_p[k][:],
                d_gp[0:1, w0:w0 + W].rearrange("one t p -> one (t p)"))
            nc.sync.dma_start(
                pt_p[k][:],
                d_gp[1:2, w0:w0 + W].rearrange("one t p -> one (t p)"))

        def stitch_in(kw):
            nonlocal carry, pcore
            k = kw % 2
            w0 = kw * W
            gt = gt_p[k]
            pt = pt_p[k]

            scw = pool.tile([1, W * P], dt.float32, tag=f"scw_{k}",
                            name=f"scw_{k}")
            nc.vector.tensor_tensor_scan(scw[:], pt[:], gt[:], carry[:],
                                         AOT.mult, AOT.max)
            seeds_row = pool.tile([1, W * P], dt.float32, tag=f"sr_{k}",
                                  name=f"sr_{k}")
            nc.vector.tensor_copy(seeds_row[0:1, 0:1], carry[:])
            nc.vector.tensor_copy(seeds_row[0:1, 1:], scw[0:1, :W * P - 1])
            nc.sync.dma_start(
                d_seed[0:1, w0:w0 + W, :].rearrange("one t p -> one (t p)"),
                seeds_row[:])

            for tt_ in range(w0, w0 + W):
                nc.sync.dma_start(
                    seeds_p[tt_ % 2][:],
                    d_seed[0, tt_:tt_ + 1, :].rearrange("one p -> p one"))

            ncarry = pool.tile([1, 1], dt.float32, tag=f"carry{kw}",
                               name=f"carry{kw}")
            nc.vector.tensor_copy(ncarry[:], scw[0:1, W * P - 1:W * P])
            carry = ncarry
            ptile = pool.tile([1, 1], dt.float32, tag=f"ptile{kw}",
                              name=f"ptile{kw}")
            nc.vector.tensor_reduce(ptile[:], pt[:], mybir.AxisListType.X,
                                    AOT.min)
            npcore = pool.tile([1, 1], dt.float32, tag=f"pcore{kw}",
                               name=f"pcore{kw}")
            nc.vector.tensor_tensor(npcore[:], pcore[:], ptile[:], AOT.min)
            pcore = npcore

        # ---------------- phase 2 ------------------------------------------
        def phase2_ops(t):
            s = t % SLOTS
            seeds = seeds_p[t % 2]

            ci = scratch("ci", t, dt.uint32, [P, f + 1])
            yield lambda: nc.vector.tensor_tensor_scan(
                ci[:, 1:f + 1], k8_t[s][:], g01_t[s][:], seeds[:],
                AOT.add, AOT.max)
            yield lambda: nc.vector.tensor_copy(ci[:, 0:1], seeds[:])

            sp = scratch("t0", t)
            yield lambda: gp_add(sp[:], S_t[s][:], ci[:, 0:f])

            a2 = a_in[t % 2]
            yield lambda: nc.sync.dma_start(a2[:], d_a[t])

            wb = pool.tile([P, f], dt.uint32, tag=f"ci_{t % 2}",
                           name=f"wb_{t % 2}")
            yield from brev_chain(t, sp, wb)

            t1 = scratch("A", t)
            yield lambda: tt(t1[:], wb[:], b_t[s][:], AOT.bitwise_xor)
            if t + 2 * W < n_tiles:    # b slot free: prefetch wave kw+2
                yield lambda: nc.sync.dma_start(b_t[s][:], d_b[t + 2 * W])
            o_t = scratch("t0", t)
            yield lambda: tt(o_t[:], t1[:], a2[:], AOT.bitwise_or)
            if t + 2 * W < n_tiles:    # a scratch free: prefetch wave kw+2
                yield lambda: nc.sync.dma_start(a_in[t % 2][:],
                                                d_a[t + 2 * W])
            yield lambda: nc.sync.dma_start(d_o[t], o_t[:])

        # ---------------- pipelined emission -------------------------------
        def emit_pairs(gen_fn, kw, skew=7):
            # Skewed pairwise interleave: tile t runs `skew` ops ahead of
            # t+1 so their GPSIMD adds don't align (DVE fills the gap).
            for t in range(kw * W, (kw + 1) * W, 2):
                gens = [gen_fn(t), gen_fn(t + 1)]
                done = [False, False]
                for _ in range(skew):
                    try:
                        next(gens[0])()
                    except StopIteration:
                        done[0] = True
                        break
                while not all(done):
                    for i, g in enumerate(gens):
                        if not done[i]:
                            try:
                                next(g)()
                            except StopIteration:
                                done[i] = True

        emit_pairs(phase1_ops, 0)
        stitch_out(0)
        emit_pairs(phase1_ops, 1)
        stitch_out(1)
        stitch_in(0)
        for kw in range(n_waves):
            emit_pairs(phase2_ops, kw, skew=0)
            if kw + 2 < n_waves:
                emit_pairs(phase1_ops, kw + 2)
                stitch_out(kw + 2)
            if kw + 1 < n_waves:
                stitch_in(kw + 1)

        agg = pool.tile([1, 2], dt.float32, tag="agg")
        nc.vector.tensor_copy(agg[0:1, 0:1], carry[:])
        nc.vector.tensor_copy(agg[0:1, 1:2], pcore[:])
        nc.sync.dma_start(d_agg[:], agg[:])

    nc.compile()
    return nc


def _get_nc(n_tiles, f):
    key = (n_tiles, f)
    if key not in _cache:
        _cache[key] = _build(n_tiles, f)
    return _cache[key]


def run_sharded(a_u8, b_u8, n_cores=N_CORES, f=F, **spmd_kwargs):
    from concourse import bass_utils

    n = a_u8.size
    words = n // 4
    wpc = words // n_cores
    n_tiles = wpc // (P * f)
    assert n_tiles * P * f == wpc, (n, n_cores, f)

    a32 = a_u8.view(np.uint32).reshape(n_cores, n_tiles, P, f)
    b32 = b_u8.view(np.uint32).reshape(n_cores, n_tiles, P, f)

    nc = _get_nc(n_tiles, f)
    in_maps = [{"a": np.ascontiguousarray(a32[c]),
                "b": np.ascontiguousarray(b32[c])}
               for c in range(n_cores)]
    res = bass_utils.run_bass_kernel_spmd(nc, in_maps,
                                          core_ids=list(range(n_cores)),
                                          **spmd_kwargs)
    outs = [r["o"] for r in res.results]
    aggs = [(float(r["agg"][0, 0]), float(r["agg"][0, 1]))
            for r in res.results]
    out = np.concatenate([o.reshape(-1).view(np.uint8) for o in outs])
    return out, aggs, res


def _fixup_boundaries(out, a_u8, b_u8, aggs, n_cores):
    """Resolve the cross-core carry (decoupled lookback, host side)."""
    n = out.size
    csize = n // n_cores
    c_in = 0
    for c in range(n_cores):
        g_c = 1 if aggs[c][0] != 0.0 else 0
        p_c = 1 if aggs[c][1] != 0.0 else 0
        if c_in == 1:
            st = c * csize
            en = st + csize
            i = st
            done = False
            while i < en and not done:
                j = min(i + 65536, en)
                aa = a_u8[i:j]
                bb = b_u8[i:j]
                raw = _BREV[aa & bb].astype(np.int32) + _BREV[bb].astype(np.int32)
                prop = raw == 255
                if prop.all():
                    out[i:j] = aa | bb
                    i = j
                    continue
                k = int(np.argmin(prop))
                out[i:i + k] = aa[:k] | bb[:k]
                idx = i + k
                new_s = (int(raw[k]) + 1) & 0xFF
                out[idx] = (int(_BREV[new_s]) ^ int(b_u8[idx])) | int(a_u8[idx])
                done = True
        c_in = g_c | (p_c & c_in)
    return out


def kernel(a, b):
    assert a.dtype == np.uint8 and b.dtype == np.uint8 and a.size == N_BYTES
    out, aggs, _ = run_sharded(a, b)
    out = _fixup_boundaries(out, a, b, aggs, N_CORES)
    return out
